# revision 8
# baseline (speedup 1.0000x reference)
# Self-contained Trainium2 Bass kernel for the LN->QKV->sparse-rel-pos-attention->proj block.
#
# Reference computation (B=128, N=256, DIM=512, H=12, KD=32, D=128):
#   xn   = LayerNorm(x) * gamma + beta
#   qkv  = xn @ Wqkv + bqkv ; split q,k,v per head
#   attn = softmax(q k^T / sqrt(KD) + biases[:, bias_idxs])
#   out  = (attn @ v) @ Wproj + bproj
#
# Strategy: pure data-parallel over batch across 8 NeuronCores (16 elems/core).
# Host folds: gamma/beta into Wqkv, 1/sqrt(KD) into Wq, v-bias into bproj,
# and expands exp(biases[:, bias_idxs]) so softmax(S+B) = expS*expB row-normalized.
# Device layouts avoid all transposes except the initial z -> z^T:
#   qk^T [feat, tok] and v [tok, feat] both come from matmuls against z^T;
#   S^T = k q^T has tokens-m on partitions so exp/Z/AV consume it directly;
#   AV gives O^T [head-dim, tok] which is exactly proj's stationary layout.
# Softmax normalizer: Z is computed partition-broadcast directly by a matmul
# with an all-ones [128,128] stationary (same PE cost as a [1,x] row, but the
# result lands replicated across partitions -> no DRAM-roundtrip broadcast),
# then one DVE reciprocal and one fused DVE multiply.
# q/k features pack into 8 chunks of 128 (strips of 32 at bases 0/32/64 --
# the PE requires stationary base partition in {0,32,64}): head h has q in
# chunk h//3, k in chunk 4+h//3, both at strip rows 32*(h%3). Heads are
# processed in strip-pure pairs because interleaving different PE
# tile_positions on one PSUM bank hangs the device.
# Transposes run all-f32r (1.5 cyc/row vs 2.0 for plain fp32; the compiler
# rejects mixing 32-bit and 16-bit matmul operands).

import numpy as np

B, N, DIM = 128, 256, 512
H, KD = 12, 32
D = 128
DH = D * H
RES = 16
EPS = 1e-5
NCORES = 8
BPC = B // NCORES

_CACHE = {}

# heads processed in strip-pure pairs: strips (h % 3) equal within each pair
HEAD_ORDER = [0, 3, 6, 9, 1, 4, 7, 10, 2, 5, 8, 11]
# rounds as (slot_offset, n_heads)
ROUNDS = [(0, 2), (2, 2), (4, 2), (6, 2), (8, 2), (10, 2)]


def _build(bpc, use_bqk, use_bp):
    from contextlib import ExitStack

    import concourse.bacc as bacc
    import concourse.tile as tile
    from concourse import mybir

    f32 = mybir.dt.float32
    f32r = mybir.dt.float32r
    Alu = mybir.AluOpType
    Act = mybir.ActivationFunctionType

    nc = bacc.Bacc("TRN2", target_bir_lowering=False, debug=False,
                   num_devices=NCORES)

    x_d = nc.dram_tensor("x", [bpc, N, DIM], f32r, kind="ExternalInput").ap()
    wqk_d = nc.dram_tensor("wqk", [DIM, 8 * 128], f32r, kind="ExternalInput").ap()
    wv_d = nc.dram_tensor("wv", [DIM, DH], f32r, kind="ExternalInput").ap()
    wp_d = nc.dram_tensor("wp", [DH, DIM], f32r, kind="ExternalInput").ap()
    expb_d = nc.dram_tensor("expb", [128, 2, H, N], f32, kind="ExternalInput").ap()
    ones_d = nc.dram_tensor("ones", [128, 128], f32r, kind="ExternalInput").ap()
    ident_d = nc.dram_tensor("ident", [128, 128], f32r, kind="ExternalInput").ap()
    if use_bqk:
        bqk_d = nc.dram_tensor("bqk", [128, 8], f32, kind="ExternalInput").ap()
    if use_bp:
        bp_d = nc.dram_tensor("bp", [DIM], f32, kind="ExternalInput").ap()
    y_d = nc.dram_tensor("y", [bpc, N, DIM], f32, kind="ExternalOutput").ap()

    with tile.TileContext(nc) as tc, ExitStack() as ctx:
        consts = ctx.enter_context(tc.tile_pool(name="consts", bufs=1))
        sb_x = ctx.enter_context(tc.tile_pool(name="sb_x", bufs=2))
        sb_zT = ctx.enter_context(tc.tile_pool(name="sb_zT", bufs=2))
        sb_qkT = ctx.enter_context(tc.tile_pool(name="sb_qkT", bufs=2))
        sb_v = ctx.enter_context(tc.tile_pool(name="sb_v", bufs=2))
        sb_pt = ctx.enter_context(tc.tile_pool(name="sb_pt", bufs=2))
        sb_zb = ctx.enter_context(tc.tile_pool(name="sb_zb", bufs=2))
        sb_ot = ctx.enter_context(tc.tile_pool(name="sb_ot", bufs=2))
        sb_small = ctx.enter_context(tc.tile_pool(name="sb_small", bufs=3))
        ps_work = ctx.enter_context(tc.tile_pool(name="ps_work", bufs=2, space="PSUM"))
        ps_s = ctx.enter_context(tc.tile_pool(name="ps_s", bufs=2, space="PSUM"))
        ps_ot = ctx.enter_context(tc.tile_pool(name="ps_ot", bufs=2, space="PSUM"))
        ps_z = ctx.enter_context(tc.tile_pool(name="ps_z", bufs=1, space="PSUM"))
        ps_y = ctx.enter_context(tc.tile_pool(name="ps_y", bufs=1, space="PSUM"))

        # ---- x for the first pair goes out on the sync queue before the
        # (much larger) consts, which ride the Activation DMA queue, so LN
        # work starts ~3us in instead of waiting on ~11MB of weights.
        x_pre = []
        for el in range(2):
            for tci in range(2):
                x_t = sb_x.tile([128, DIM], f32r, tag="x", bufs=5)
                nc.sync.dma_start(out=x_t,
                                  in_=x_d[el, tci * 128:(tci + 1) * 128, :])
                x_pre.append(x_t)

        # ---- constants (ordered by first use) ----
        wqk_sb = consts.tile([128, 4, 8 * 128], f32r)
        nc.scalar.dma_start(out=wqk_sb, in_=wqk_d.rearrange("(kc p) f -> p kc f", p=128))
        wv_sb = consts.tile([128, 4, DH], f32r)
        nc.scalar.dma_start(out=wv_sb, in_=wv_d.rearrange("(kc p) f -> p kc f", p=128))
        expb_sb = consts.tile([128, 2, H, N], f32)
        nc.scalar.dma_start(out=expb_sb, in_=expb_d)
        wp_sb = consts.tile([128, H, DIM], f32r)
        nc.scalar.dma_start(out=wp_sb, in_=wp_d.rearrange("(h p) f -> p h f", p=128))
        ones_sb = consts.tile([128, 128], f32r)
        nc.scalar.dma_start(out=ones_sb, in_=ones_d)
        ident = consts.tile([128, 128], f32r)
        nc.scalar.dma_start(out=ident, in_=ident_d)
        eps_t = consts.tile([128, 1], f32)
        nc.vector.memset(eps_t, EPS)
        if use_bqk:
            bqk_sb = consts.tile([128, 8], f32)
            nc.scalar.dma_start(out=bqk_sb, in_=bqk_d)
        if use_bp:
            bp_sb = consts.tile([128, 1, DIM], f32)
            nc.scalar.dma_start(out=bp_sb, in_=bp_d.partition_broadcast(128))

        assert bpc % 2 == 0
        for ep in range(bpc // 2):
            # ---- LayerNorm (token-major) + PE transpose to z^T, elem pair --
            # all four bn-stats first so ACT runs a single Sqrt per pair
            # (table reloads between Sqrt and Exp are ~1.3us each)
            zT_sb = sb_zT.tile([128, 4, 2 * N], f32r, tag="zT")
            x_ts = []
            mv = sb_small.tile([128, 2, 2, 2], f32, tag="mv")
            for el in range(2):
                for tci in range(2):
                    if ep == 0:
                        x_t = x_pre[2 * el + tci]
                    else:
                        x_t = sb_x.tile([128, DIM], f32r, tag="x", bufs=5)
                        nc.sync.dma_start(
                            out=x_t,
                            in_=x_d[2 * ep + el, tci * 128:(tci + 1) * 128, :])
                    stats = sb_small.tile([128, 6], f32, tag="stats")
                    nc.vector.bn_stats(stats, x_t)
                    nc.vector.bn_aggr(mv[:, el, tci, :], stats)
                    x_ts.append(x_t)
            sig = sb_small.tile([128, 2, 2], f32, tag="sig")
            nc.scalar.activation(sig, mv[:, :, :, 1], Act.Sqrt, bias=eps_t,
                                 scale=1.0)
            rsig = sb_small.tile([128, 2, 2], f32, tag="rsig")
            nc.vector.reciprocal(rsig, sig)
            for el in range(2):
                for tci in range(2):
                    x_t = x_ts[2 * el + tci]
                    nc.vector.tensor_scalar(out=x_t, in0=x_t,
                                            scalar1=mv[:, el, tci, 0:1],
                                            scalar2=rsig[:, el, tci:tci + 1],
                                            op0=Alu.subtract, op1=Alu.mult)
                    zT_ps = ps_work.tile([128, 512], f32r, tag="work")
                    for kc in range(4):
                        nc.tensor.transpose(zT_ps[:, kc * 128:(kc + 1) * 128],
                                            x_t[:, kc * 128:(kc + 1) * 128],
                                            ident)
                    off = el * N + tci * 128
                    nc.scalar.activation(zT_sb[:, :, off:off + 128],
                                         zT_ps.rearrange("p (kc t) -> p kc t",
                                                         kc=4),
                                         Act.Copy)

            # ---- qk^T = W'' ^T z^T   [feat, tok-pair].  Head h's q lives in
            # chunk h//3, its k in chunk 4 + h//3, both at 32-row strip h%3.
            qkT_sb = sb_qkT.tile([128, 8, 2 * N], f32r, tag="qkT", bufs=1)
            for fc in range(8):
                qk_ps = ps_work.tile([128, 512], f32, tag="work")
                for kc in range(4):
                    nc.tensor.matmul(qk_ps,
                                     lhsT=wqk_sb[:, kc, fc * 128:(fc + 1) * 128],
                                     rhs=zT_sb[:, kc, :],
                                     start=(kc == 0), stop=(kc == 3))
                nc.scalar.activation(qkT_sb[:, fc, :], qk_ps, Act.Copy)
                if use_bqk:
                    nc.vector.tensor_scalar_add(
                        out=qkT_sb[:, fc, :], in0=qkT_sb[:, fc, :],
                        scalar1=bqk_sb[:, fc:fc + 1])

            for el in range(2):
                e = 2 * ep + el
                etok = el * N
                # ---- v = z Wv   [tok 256, feat 1536] ----
                v_sb = sb_v.tile([128, 2, DH], f32r, tag="v")
                for mc in range(2):
                    for ns in range(3):
                        v_ps = ps_work.tile([128, 512], f32, tag="work")
                        for kc in range(4):
                            nc.tensor.matmul(
                                v_ps,
                                lhsT=zT_sb[:, kc,
                                           etok + mc * 128:etok + (mc + 1) * 128],
                                rhs=wv_sb[:, kc, ns * 512:(ns + 1) * 512],
                                start=(kc == 0), stop=(kc == 3))
                        nc.vector.tensor_copy(
                            out=v_sb[:, mc, ns * 512:(ns + 1) * 512], in_=v_ps)

                # ---- attention, strip-pure rounds (4 pairs + 4 singles) ----
                # slot s in pt/ot/expb corresponds to head HEAD_ORDER[s]
                ot_sb = sb_ot.tile([128, H, N], f32r, tag="ot")
                for r, (s0, nh) in enumerate(ROUNDS):
                    W = nh * N
                    pt_sb = sb_pt.tile([128, 2, nh, N], f32r, tag="pt")
                    for mc in range(2):
                        s_ps = ps_s.tile([128, 512], f32, tag="s")
                        for hi in range(nh):
                            h = HEAD_ORDER[s0 + hi]
                            qc = h // 3
                            base = (h % 3) * KD
                            nc.tensor.matmul(
                                s_ps[:, hi * N:(hi + 1) * N],
                                lhsT=qkT_sb[base:base + KD, 4 + qc,
                                            etok + mc * 128:etok + (mc + 1) * 128],
                                rhs=qkT_sb[base:base + KD, qc, etok:etok + N],
                                start=True, stop=True)
                        nc.scalar.activation(pt_sb[:, mc],
                                             s_ps[:, :W].rearrange(
                                                 "p (a n) -> p a n", a=nh),
                                             Act.Exp)
                        # alternate the expB multiply between GpSimd and DVE
                        eng = nc.gpsimd if (r + mc) % 2 == 0 else nc.vector
                        eng.tensor_tensor(out=pt_sb[:, mc], in0=pt_sb[:, mc],
                                          in1=expb_sb[:, mc, s0:s0 + nh, :],
                                          op=Alu.mult)
                    # Z[n] = sum_m P^T[m, n], partition-broadcast via all-ones
                    # stationary (same PE cost as a 1-row output, no DMA)
                    zb_ps = ps_z.tile([128, 512], f32, tag="zb")
                    for mc in range(2):
                        nc.tensor.matmul(zb_ps[:, :W],
                                         lhsT=ones_sb,
                                         rhs=pt_sb[:, mc, :, :].rearrange(
                                             "p a n -> p (a n)"),
                                         start=(mc == 0), stop=(mc == 1))
                    zr_sb = sb_zb.tile([128, nh, N], f32, tag="zb")
                    nc.vector.reciprocal(
                        zr_sb, zb_ps[:, :W].rearrange("p (a n) -> p a n", a=nh))
                    # O^T = v^T P^T  (normalized by zr afterwards)
                    ot_ps = ps_ot.tile([128, 512], f32, tag="otp")
                    for hi in range(nh):
                        h = HEAD_ORDER[s0 + hi]
                        for mc in range(2):
                            nc.tensor.matmul(
                                ot_ps[:, hi * N:(hi + 1) * N],
                                lhsT=v_sb[:, mc, h * 128:(h + 1) * 128],
                                rhs=pt_sb[:, mc, hi, :],
                                start=(mc == 0), stop=(mc == 1))
                    nc.vector.tensor_tensor(
                        out=ot_sb[:, s0:s0 + nh, :],
                        in0=ot_ps[:, :W].rearrange("p (a n) -> p a n", a=nh),
                        in1=zr_sb, op=Alu.mult)

                # ---- proj: y = O Wp ----
                for nci in range(2):
                    y_ps = ps_y.tile([128, 512], f32, tag="y")
                    for slot in range(H):
                        nc.tensor.matmul(y_ps,
                                         lhsT=ot_sb[:, slot, nci * 128:(nci + 1) * 128],
                                         rhs=wp_sb[:, HEAD_ORDER[slot], :],
                                         start=(slot == 0), stop=(slot == H - 1))
                    yb_sb = sb_x.tile([128, DIM], f32, tag="yb")
                    if use_bp:
                        nc.vector.tensor_tensor(out=yb_sb, in0=y_ps, in1=bp_sb[:, 0, :],
                                                op=Alu.add)
                    else:
                        nc.scalar.activation(yb_sb, y_ps, Act.Copy)
                    nc.sync.dma_start(out=y_d[e, nci * 128:(nci + 1) * 128, :],
                                      in_=yb_sb)

    nc.compile()
    return nc


def _prepare(x, gamma, beta, Wqkv, bqkv, Wproj, bproj, biases, bias_idxs):
    x = np.ascontiguousarray(np.asarray(x, dtype=np.float32))
    gamma = np.asarray(gamma, dtype=np.float32)
    beta = np.asarray(beta, dtype=np.float32)
    Wqkv = np.asarray(Wqkv, dtype=np.float32)
    bqkv = np.asarray(bqkv, dtype=np.float32)
    Wproj = np.asarray(Wproj, dtype=np.float32)
    bproj = np.asarray(bproj, dtype=np.float32)
    biases = np.asarray(biases, dtype=np.float32)
    bias_idxs = np.asarray(bias_idxs)

    s = np.float32(KD ** -0.5)
    Wg = Wqkv * gamma[:, None]
    bfull = beta @ Wqkv + bqkv
    Wr = Wg.reshape(DIM, H, 64 + D)
    br = bfull.reshape(H, 64 + D)
    # feature layout (see kernel comment): head h -> strip h%3; q in chunk
    # h//3, k in chunk 4 + h//3.
    wqk = np.zeros((DIM, 8, 128), dtype=np.float32)
    bqk = np.zeros((8, 128), dtype=np.float32)
    for h in range(H):
        qc, base = h // 3, (h % 3) * KD
        wqk[:, qc, base:base + KD] = Wr[:, h, 0:KD] * s
        wqk[:, 4 + qc, base:base + KD] = Wr[:, h, KD:2 * KD]
        bqk[qc, base:base + KD] = br[h, 0:KD] * s
        bqk[4 + qc, base:base + KD] = br[h, KD:2 * KD]
    wqk = np.ascontiguousarray(wqk.reshape(DIM, 8 * 128))
    wv = np.ascontiguousarray(Wr[:, :, 2 * KD:].reshape(DIM, DH))
    bv = br[:, 2 * KD:].reshape(DH)
    bp = bproj + bv @ Wproj
    expb = np.exp(biases[:, bias_idxs])  # [H, N, N]
    # head dim reordered to the kernel's processing order
    expb_t = np.ascontiguousarray(
        expb[HEAD_ORDER].reshape(H, 2, 128, N).transpose(2, 1, 0, 3))

    use_bqk = bool(np.abs(bqk).max() > 0)
    use_bp = bool(np.abs(bp).max() > 0)
    bqk_t = np.ascontiguousarray(bqk.T)  # [128, 8]

    common = {"wqk": wqk, "wv": wv, "wp": np.ascontiguousarray(Wproj),
              "expb": expb_t, "ones": np.ones((128, 128), dtype=np.float32),
              "ident": np.eye(128, dtype=np.float32)}
    if use_bqk:
        common["bqk"] = bqk_t
    if use_bp:
        common["bp"] = np.ascontiguousarray(bp)
    in_maps = []
    for c in range(NCORES):
        m = dict(common)
        m["x"] = np.ascontiguousarray(x[c * BPC:(c + 1) * BPC])
        in_maps.append(m)
    return in_maps, use_bqk, use_bp


def run(inputs, trace=False, **run_kwargs):
    from concourse.bass_utils import run_bass_kernel_spmd

    in_maps, use_bqk, use_bp = _prepare(**inputs)
    key = (BPC, use_bqk, use_bp)
    if key not in _CACHE:
        _CACHE[key] = _build(*key)
    nc = _CACHE[key]
    res = run_bass_kernel_spmd(nc, in_maps, core_ids=list(range(NCORES)),
                               trace=trace, **run_kwargs)
    y = np.concatenate([res.results[c]["y"] for c in range(NCORES)], axis=0)
    return y, res


def kernel(**inputs):
    y, _ = run(inputs)
    return y


# revision 11
# speedup vs baseline: 1.4704x; 1.4704x over previous
# Self-contained Trainium2 Bass kernel for the LN->QKV->sparse-rel-pos-attention->proj block.
#
# Reference computation (B=128, N=256, DIM=512, H=12, KD=32, D=128):
#   xn   = LayerNorm(x) * gamma + beta
#   qkv  = xn @ Wqkv + bqkv ; split q,k,v per head
#   attn = softmax(q k^T / sqrt(KD) + biases[:, bias_idxs])
#   out  = (attn @ v) @ Wproj + bproj
#
# Strategy: pure data-parallel over batch across 8 NeuronCores (16 elems/core).
# Host folds: gamma/beta into Wqkv, 1/sqrt(KD) into Wq, v-bias into bproj,
# and expands exp(biases[:, bias_idxs]) so softmax(S+B) = expS*expB row-normalized.
# Device layouts avoid all transposes except the initial z -> z^T:
#   qk^T [feat, tok] and v [tok, feat] both come from matmuls against z^T;
#   S^T = k q^T has tokens-m on partitions so exp/Z/AV consume it directly;
#   AV gives O^T [head-dim, tok] which is exactly proj's stationary layout.
# Softmax normalizer: Z is computed partition-broadcast directly by a matmul
# with an all-ones [128,128] stationary (same PE cost as a [1,x] row, but the
# result lands replicated across partitions -> no DRAM-roundtrip broadcast),
# then one DVE approx-reciprocal and one fused multiply.
# q/k features pack into 8 chunks of 128 (strips of 32 at bases 0/32/64 --
# the PE requires stationary base partition in {0,32,64}): head h has q in
# chunk h//3, k in chunk 4+h//3, both at strip rows 32*(h%3). Heads are
# processed in strip-pure pairs because interleaving different PE
# tile_positions on one PSUM bank hangs the device.
# PE p-state: full speed (2.4GHz) only after 3us of continuous busy; every
# idle gap drops it to 1.2GHz for the next 3us. The emission order is
# software-pipelined so the PE rarely waits: v-production matmuls interleave
# with S rounds, Z/AV trail S by 2 rounds (covering the exp->expB-mult
# latency on ACT/DVE/GpSimd), next-pair LayerNorm DVE work is spread through
# this pair's rounds, and next-pair transposes cover proj's wait for the
# last normalize.
# x/y ride DMA with 2 tokens per partition line (4KB contiguous packets;
# token-major 2KB lines only reach ~23GB/s) => attention token order is
# position (s*128+q) <-> token 2q+s; expb is permuted to match on the host.
# Weight constants ride the Activation-engine DMA queue in first-use order
# so the first x tiles are not stuck behind ~9MB of weights.

import numpy as np

B, N, DIM = 128, 256, 512
H, KD = 12, 32
D = 128
DH = D * H
RES = 16
EPS = 1e-5
NCORES = 8
BPC = B // NCORES

_CACHE = {}

# heads processed in strip-pure pairs ((h%3) equal within each pair), ordered
# so v chunks (c0: heads 0-3, c1: 4-7, c2: 8-11) are needed progressively
HEAD_ORDER = [0, 3, 1, 4, 2, 5, 6, 9, 7, 10, 8, 11]
# qk chunk emission order: rounds 0-2 need q chunks {0,1}, k {4,5}
FC_ORDER = [0, 1, 4, 5, 2, 3, 6, 7]


def _build(bpc, use_bqk, use_bp):
    from contextlib import ExitStack

    import concourse.bacc as bacc
    import concourse.tile as tile
    from concourse import mybir

    f32 = mybir.dt.float32
    f32r = mybir.dt.float32r
    fp16 = mybir.dt.float16
    Alu = mybir.AluOpType
    Act = mybir.ActivationFunctionType

    nc = bacc.Bacc("TRN2", target_bir_lowering=False, debug=False,
                   num_devices=NCORES)

    x_d = nc.dram_tensor("x", [bpc, N, DIM], f32r, kind="ExternalInput").ap()
    wqk_d = nc.dram_tensor("wqk", [DIM, 8 * 128], f32r, kind="ExternalInput").ap()
    wv_d = nc.dram_tensor("wv", [DIM, DH], f32r, kind="ExternalInput").ap()
    wp_d = nc.dram_tensor("wp", [DH, DIM], f32r, kind="ExternalInput").ap()
    expb_d = nc.dram_tensor("expb", [128, 2, H, N], fp16, kind="ExternalInput").ap()
    ones_d = nc.dram_tensor("ones", [128, 128], f32r, kind="ExternalInput").ap()
    ident_d = nc.dram_tensor("ident", [128, 128], f32r, kind="ExternalInput").ap()
    if use_bqk:
        bqk_d = nc.dram_tensor("bqk", [128, 8], f32, kind="ExternalInput").ap()
    if use_bp:
        bp_d = nc.dram_tensor("bp", [DIM], f32, kind="ExternalInput").ap()
    y_d = nc.dram_tensor("y", [bpc, N, DIM], f32, kind="ExternalOutput").ap()

    NP = bpc // 2

    with tile.TileContext(nc) as tc, ExitStack() as ctx:
        consts = ctx.enter_context(tc.tile_pool(name="consts", bufs=1))
        sb_x = ctx.enter_context(tc.tile_pool(name="sb_x", bufs=2))
        sb_zT = ctx.enter_context(tc.tile_pool(name="sb_zT", bufs=2))
        sb_qkT = ctx.enter_context(tc.tile_pool(name="sb_qkT", bufs=2))
        sb_v = ctx.enter_context(tc.tile_pool(name="sb_v", bufs=2))
        sb_pt = ctx.enter_context(tc.tile_pool(name="sb_pt", bufs=3))
        sb_zb = ctx.enter_context(tc.tile_pool(name="sb_zb", bufs=2))
        sb_ot = ctx.enter_context(tc.tile_pool(name="sb_ot", bufs=2))
        sb_yb = ctx.enter_context(tc.tile_pool(name="sb_yb", bufs=2))
        sb_small = ctx.enter_context(tc.tile_pool(name="sb_small", bufs=3))
        ps_work = ctx.enter_context(tc.tile_pool(name="ps_work", bufs=2, space="PSUM"))
        ps_s = ctx.enter_context(tc.tile_pool(name="ps_s", bufs=2, space="PSUM"))
        ps_ot = ctx.enter_context(tc.tile_pool(name="ps_ot", bufs=2, space="PSUM"))
        ps_z = ctx.enter_context(tc.tile_pool(name="ps_z", bufs=2, space="PSUM"))

        x_tiles = {}

        def issue_x(p):
            t = sb_x.tile([128, 4, DIM], f32r, tag="x", bufs=2)
            nc.sync.dma_start(
                out=t.rearrange("q (e two) d -> q e two d", e=2),
                in_=x_d[2 * p:2 * p + 2].rearrange("e (q two) d -> q e two d",
                                                   q=128))
            x_tiles[p] = t

        # first pair's x before the big consts
        issue_x(0)

        # ---- constants (Activation DMA queue, first-use order) ----
        ident = consts.tile([128, 128], f32r)
        nc.scalar.dma_start(out=ident, in_=ident_d)
        wqk_sb = consts.tile([128, 4, 8 * 128], f32r)
        nc.scalar.dma_start(out=wqk_sb, in_=wqk_d.rearrange("(kc p) f -> p kc f", p=128))
        wv_sb = consts.tile([128, 4, DH], f32r)
        nc.scalar.dma_start(out=wv_sb, in_=wv_d.rearrange("(kc p) f -> p kc f", p=128))
        expb_sb = consts.tile([128, 2, H, N], fp16)
        nc.scalar.dma_start(out=expb_sb, in_=expb_d)
        ones_sb = consts.tile([128, 128], f32r)
        nc.scalar.dma_start(out=ones_sb, in_=ones_d)
        wp_sb = consts.tile([128, H, DIM], f32r)
        nc.scalar.dma_start(out=wp_sb, in_=wp_d.rearrange("(h p) f -> p h f", p=128))
        eps_t = consts.tile([128, 1], f32)
        nc.vector.memset(eps_t, EPS)
        if use_bqk:
            bqk_sb = consts.tile([128, 8], f32)
            nc.scalar.dma_start(out=bqk_sb, in_=bqk_d)
        if use_bp:
            bp_sb = consts.tile([128, 1, DIM], f32)
            nc.scalar.dma_start(out=bp_sb, in_=bp_d.partition_broadcast(128))

        # ---- LayerNorm pieces (emitted spread through the previous pair) ----
        def ln_stats(p, st, mvt):
            stats = sb_small.tile([128, 6], f32, tag="stats", bufs=5)
            nc.vector.bn_stats(stats, x_tiles[p][:, st, :])
            nc.vector.bn_aggr(mvt[:, st, :], stats)

        def ln_finish(mvt):
            sig = sb_small.tile([128, 4], f32, tag="sig")
            nc.scalar.activation(sig, mvt[:, :, 1], Act.Sqrt, bias=eps_t,
                                 scale=1.0)
            rsig = sb_small.tile([128, 4], f32, tag="rsig")
            nc.vector.reciprocal(rsig, sig)
            return rsig

        def ln_norm(p, st, mvt, rsig):
            x_t = x_tiles[p]
            nc.vector.tensor_scalar(out=x_t[:, st, :], in0=x_t[:, st, :],
                                    scalar1=mvt[:, st, 0:1],
                                    scalar2=rsig[:, st:st + 1],
                                    op0=Alu.subtract, op1=Alu.mult)

        def transposes(p):
            zT = sb_zT.tile([128, 4, 2 * N], f32r, tag="zT")
            x_t = x_tiles[p]
            for st in range(4):
                el, sl = st // 2, st % 2
                zT_ps = ps_work.tile([128, 512], f32r, tag="work")
                for kc in range(4):
                    nc.tensor.transpose(zT_ps[:, kc * 128:(kc + 1) * 128],
                                        x_t[:, st, kc * 128:(kc + 1) * 128],
                                        ident)
                off = el * N + sl * 128
                nc.scalar.activation(zT[:, :, off:off + 128],
                                     zT_ps.rearrange("p (kc t) -> p kc t", kc=4),
                                     Act.Copy)
            return zT

        def qk_prod(zT):
            qkT = sb_qkT.tile([128, 8, 2 * N], f32r, tag="qkT", bufs=1)
            for i, fc in enumerate(FC_ORDER):
                qk_ps = ps_work.tile([128, 512], f32, tag="work")
                for kc in range(4):
                    nc.tensor.matmul(qk_ps,
                                     lhsT=wqk_sb[:, kc, fc * 128:(fc + 1) * 128],
                                     rhs=zT[:, kc, :],
                                     start=(kc == 0), stop=(kc == 3))
                if i % 2 == 0:
                    nc.scalar.activation(qkT[:, fc, :], qk_ps, Act.Copy)
                else:
                    nc.vector.tensor_copy(out=qkT[:, fc, :], in_=qk_ps)
                if use_bqk:
                    nc.vector.tensor_scalar_add(
                        out=qkT[:, fc, :], in0=qkT[:, fc, :],
                        scalar1=bqk_sb[:, fc:fc + 1])
            return qkT

        def v_mm(zT, v_sb, etok, mc, c):
            v_ps = ps_work.tile([128, 512], f32, tag="work")
            for kc in range(4):
                nc.tensor.matmul(
                    v_ps,
                    lhsT=zT[:, kc, etok + mc * 128:etok + (mc + 1) * 128],
                    rhs=wv_sb[:, kc, c * 512:(c + 1) * 512],
                    start=(kc == 0), stop=(kc == 3))
            nc.scalar.activation(v_sb[:, mc, c * 512:(c + 1) * 512], v_ps,
                                 Act.Copy)

        def s_round(qkT, etok, r):
            pt = sb_pt.tile([128, 2, 2, N], f32r, tag="pt")
            for mc in range(2):
                s_ps = ps_s.tile([128, 512], f32, tag="s")
                for hi in range(2):
                    h = HEAD_ORDER[2 * r + hi]
                    qc = h // 3
                    base = (h % 3) * KD
                    nc.tensor.matmul(
                        s_ps[:, hi * N:(hi + 1) * N],
                        lhsT=qkT[base:base + KD, 4 + qc,
                                 etok + mc * 128:etok + (mc + 1) * 128],
                        rhs=qkT[base:base + KD, qc, etok:etok + N],
                        start=True, stop=True)
                nc.scalar.activation(pt[:, mc],
                                     s_ps.rearrange("p (a n) -> p a n", a=2),
                                     Act.Exp)
                eng = nc.gpsimd if (r + mc) % 2 == 0 else nc.vector
                eng.tensor_tensor(out=pt[:, mc], in0=pt[:, mc],
                                  in1=expb_sb[:, mc, 2 * r:2 * r + 2, :],
                                  op=Alu.mult)
            return pt

        def zav_round(pt, v_sb, ot_sb, r):
            zb_ps = ps_z.tile([128, 512], f32, tag="zb")
            for mc in range(2):
                nc.tensor.matmul(zb_ps,
                                 lhsT=ones_sb,
                                 rhs=pt[:, mc, :, :].rearrange("p a n -> p (a n)"),
                                 start=(mc == 0), stop=(mc == 1))
            zr = sb_zb.tile([128, 2, N], f32, tag="zb")
            nc.vector.reciprocal_approx_fast(
                out=zr, in_=zb_ps.rearrange("p (a n) -> p a n", a=2))
            ot_ps = ps_ot.tile([128, 512], f32, tag="otp")
            for hi in range(2):
                h = HEAD_ORDER[2 * r + hi]
                for mc in range(2):
                    nc.tensor.matmul(
                        ot_ps[:, hi * N:(hi + 1) * N],
                        lhsT=v_sb[:, mc, h * 128:(h + 1) * 128],
                        rhs=pt[:, mc, hi, :],
                        start=(mc == 0), stop=(mc == 1))
            # GpSimd cannot read PSUM; normalize stays on DVE
            nc.vector.tensor_tensor(out=ot_sb[:, 2 * r:2 * r + 2, :],
                              in0=ot_ps.rearrange("p (a n) -> p a n", a=2),
                              in1=zr, op=Alu.mult)

        def proj(ot_sb, e):
            yb = sb_yb.tile([128, 2, DIM], f32, tag="yb")
            for nci in range(2):
                y_ps = ps_work.tile([128, 512], f32, tag="work")
                for slot in range(H):
                    nc.tensor.matmul(
                        y_ps,
                        lhsT=ot_sb[:, slot, nci * 128:(nci + 1) * 128],
                        rhs=wp_sb[:, HEAD_ORDER[slot], :],
                        start=(slot == 0), stop=(slot == H - 1))
                if use_bp:
                    nc.vector.tensor_tensor(out=yb[:, nci, :], in0=y_ps,
                                            in1=bp_sb[:, 0, :], op=Alu.add)
                else:
                    nc.scalar.activation(yb[:, nci, :], y_ps, Act.Copy)
            nc.sync.dma_start(
                out=y_d[e].rearrange("(q two) d -> q two d", q=128), in_=yb)

        # ---- prologue: pair 0's LN + transposes ----
        assert bpc % 2 == 0
        mv0 = sb_small.tile([128, 4, 2], f32, tag="mv")
        for st in range(4):
            ln_stats(0, st, mv0)
        rs0 = ln_finish(mv0)
        for st in range(4):
            ln_norm(0, st, mv0, rs0)
        zT = transposes(0)

        for p in range(NP):
            if p + 1 < NP:
                issue_x(p + 1)
                mv_n = sb_small.tile([128, 4, 2], f32, tag="mv")
                rs_holder = [None]
            qkT = qk_prod(zT)

            prev = None  # (ot_sb, e) of el0 awaiting proj
            for el in range(2):
                e = 2 * p + el
                etok = el * N
                v_sb = sb_v.tile([128, 2, DH], f32r, tag="v")
                ot_sb = sb_ot.tile([128, H, N], f32r, tag="ot")
                pts = {}

                def hook(i):
                    # spread next pair's LN through el0's rounds
                    if el != 0 or p + 1 >= NP:
                        return
                    if i < 4:
                        ln_stats(p + 1, i, mv_n)
                    elif i == 4:
                        rs_holder[0] = ln_finish(mv_n)
                        ln_norm(p + 1, 0, mv_n, rs_holder[0])
                        ln_norm(p + 1, 1, mv_n, rs_holder[0])
                    else:
                        ln_norm(p + 1, 2, mv_n, rs_holder[0])
                        ln_norm(p + 1, 3, mv_n, rs_holder[0])

                v_mm(zT, v_sb, etok, 0, 0)
                v_mm(zT, v_sb, etok, 1, 0)
                if prev is not None:
                    proj(*prev)
                    prev = None
                pts[0] = s_round(qkT, etok, 0)
                hook(0)
                v_mm(zT, v_sb, etok, 0, 1)
                pts[1] = s_round(qkT, etok, 1)
                hook(1)
                v_mm(zT, v_sb, etok, 1, 1)
                pts[2] = s_round(qkT, etok, 2)
                zav_round(pts.pop(0), v_sb, ot_sb, 0)
                hook(2)
                v_mm(zT, v_sb, etok, 0, 2)
                pts[3] = s_round(qkT, etok, 3)
                zav_round(pts.pop(1), v_sb, ot_sb, 1)
                hook(3)
                v_mm(zT, v_sb, etok, 1, 2)
                pts[4] = s_round(qkT, etok, 4)
                zav_round(pts.pop(2), v_sb, ot_sb, 2)
                hook(4)
                pts[5] = s_round(qkT, etok, 5)
                zav_round(pts.pop(3), v_sb, ot_sb, 3)
                hook(5)
                zav_round(pts.pop(4), v_sb, ot_sb, 4)
                zav_round(pts.pop(5), v_sb, ot_sb, 5)

                if el == 0:
                    prev = (ot_sb, e)
                else:
                    # next pair's transposes cover proj's wait for the last
                    # normalize
                    if p + 1 < NP:
                        zT = transposes(p + 1)
                    proj(ot_sb, e)

    nc.compile()
    return nc


def _prepare(x, gamma, beta, Wqkv, bqkv, Wproj, bproj, biases, bias_idxs):
    x = np.ascontiguousarray(np.asarray(x, dtype=np.float32))
    gamma = np.asarray(gamma, dtype=np.float32)
    beta = np.asarray(beta, dtype=np.float32)
    Wqkv = np.asarray(Wqkv, dtype=np.float32)
    bqkv = np.asarray(bqkv, dtype=np.float32)
    Wproj = np.asarray(Wproj, dtype=np.float32)
    bproj = np.asarray(bproj, dtype=np.float32)
    biases = np.asarray(biases, dtype=np.float32)
    bias_idxs = np.asarray(bias_idxs)

    s = np.float32(KD ** -0.5)
    Wg = Wqkv * gamma[:, None]
    bfull = beta @ Wqkv + bqkv
    Wr = Wg.reshape(DIM, H, 64 + D)
    br = bfull.reshape(H, 64 + D)
    # feature layout (see kernel comment): head h -> strip h%3; q in chunk
    # h//3, k in chunk 4 + h//3.
    wqk = np.zeros((DIM, 8, 128), dtype=np.float32)
    bqk = np.zeros((8, 128), dtype=np.float32)
    for h in range(H):
        qc, base = h // 3, (h % 3) * KD
        wqk[:, qc, base:base + KD] = Wr[:, h, 0:KD] * s
        wqk[:, 4 + qc, base:base + KD] = Wr[:, h, KD:2 * KD]
        bqk[qc, base:base + KD] = br[h, 0:KD] * s
        bqk[4 + qc, base:base + KD] = br[h, KD:2 * KD]
    wqk = np.ascontiguousarray(wqk.reshape(DIM, 8 * 128))
    wv = np.ascontiguousarray(Wr[:, :, 2 * KD:].reshape(DIM, DH))
    bv = br[:, 2 * KD:].reshape(DH)
    bp = bproj + bv @ Wproj
    expb = np.exp(biases[:, bias_idxs])  # [H, N, N]
    # token positions are interleaved 2-per-partition: pos (s*128+q) <-> token
    # 2q+s; permute both attention axes to match, then reorder heads to the
    # kernel's processing order
    perm = np.arange(N).reshape(128, 2).T.reshape(-1)  # pos -> token
    expb_p = expb[HEAD_ORDER][:, perm][:, :, perm]
    expb_t = np.ascontiguousarray(
        expb_p.reshape(H, 2, 128, N).transpose(2, 1, 0, 3)).astype(np.float16)

    use_bqk = bool(np.abs(bqk).max() > 0)
    use_bp = bool(np.abs(bp).max() > 0)
    bqk_t = np.ascontiguousarray(bqk.T)  # [128, 8]

    common = {"wqk": wqk, "wv": wv, "wp": np.ascontiguousarray(Wproj),
              "expb": expb_t, "ones": np.ones((128, 128), dtype=np.float32),
              "ident": np.eye(128, dtype=np.float32)}
    if use_bqk:
        common["bqk"] = bqk_t
    if use_bp:
        common["bp"] = np.ascontiguousarray(bp)
    in_maps = []
    for c in range(NCORES):
        m = dict(common)
        m["x"] = np.ascontiguousarray(x[c * BPC:(c + 1) * BPC])
        in_maps.append(m)
    return in_maps, use_bqk, use_bp


def run(inputs, trace=False, **run_kwargs):
    from concourse.bass_utils import run_bass_kernel_spmd

    in_maps, use_bqk, use_bp = _prepare(**inputs)
    key = (BPC, use_bqk, use_bp)
    if key not in _CACHE:
        _CACHE[key] = _build(*key)
    nc = _CACHE[key]
    res = run_bass_kernel_spmd(nc, in_maps, core_ids=list(range(NCORES)),
                               trace=trace, **run_kwargs)
    y = np.concatenate([res.results[c]["y"] for c in range(NCORES)], axis=0)
    return y, res


def kernel(**inputs):
    y, _ = run(inputs)
    return y


# revision 12
# speedup vs baseline: 1.5515x; 1.0552x over previous
# Self-contained Trainium2 Bass kernel for the LN->QKV->sparse-rel-pos-attention->proj block.
#
# Reference computation (B=128, N=256, DIM=512, H=12, KD=32, D=128):
#   xn   = LayerNorm(x) * gamma + beta
#   qkv  = xn @ Wqkv + bqkv ; split q,k,v per head
#   attn = softmax(q k^T / sqrt(KD) + biases[:, bias_idxs])
#   out  = (attn @ v) @ Wproj + bproj
#
# Strategy: pure data-parallel over batch across 8 NeuronCores (16 elems/core).
# Host folds: gamma/beta into Wqkv, 1/sqrt(KD) into Wq, v-bias into bproj,
# and expands exp(biases[:, bias_idxs]) so softmax(S+B) = expS*expB row-normalized.
# Device layouts avoid all transposes except the initial z -> z^T:
#   qk^T [feat, tok] and v [tok, feat] both come from matmuls against z^T;
#   S^T = k q^T has tokens-m on partitions so exp/Z/AV consume it directly;
#   AV gives O^T [head-dim, tok] which is exactly proj's stationary layout.
# Softmax normalizer: Z is computed partition-broadcast directly by a matmul
# with an all-ones [128,128] stationary (same PE cost as a [1,x] row, but the
# result lands replicated across partitions -> no DRAM-roundtrip broadcast),
# then one DVE approx-reciprocal and one fused multiply.
# q/k features pack into 8 chunks of 128 (strips of 32 at bases 0/32/64 --
# the PE requires stationary base partition in {0,32,64}): head h has q in
# chunk h//3, k in chunk 4+h//3, both at strip rows 32*(h%3). Heads are
# processed in strip-pure pairs because interleaving different PE
# tile_positions on one PSUM bank hangs the device.
# PE p-state: full speed (2.4GHz) only after 3us of continuous busy; every
# idle gap drops it to 1.2GHz for the next 3us. The emission order is
# software-pipelined so the PE rarely waits: v-production matmuls interleave
# with S rounds, Z/AV trail S by 2 rounds (covering the exp->expB-mult
# latency on ACT/DVE/GpSimd), next-pair LayerNorm DVE work is spread through
# this pair's rounds, and next-pair transposes cover proj's wait for the
# last normalize.
# x/y ride DMA with 2 tokens per partition line (4KB contiguous packets;
# token-major 2KB lines only reach ~23GB/s) => attention token order is
# position (s*128+q) <-> token 2q+s; expb is permuted to match on the host.
# Weight-constant DMA descriptors are issued from otherwise-idle engines
# (ident/wqk on Activation, the rest on GpSimd; x on Sync) -- issuing one
# rearranged weight DMA costs ~3.6us of the issuing engine's queue, which
# must not block the first LayerNorm Sqrt / zT copies.

import numpy as np

B, N, DIM = 128, 256, 512
H, KD = 12, 32
D = 128
DH = D * H
RES = 16
EPS = 1e-5
NCORES = 8
BPC = B // NCORES

_CACHE = {}

# heads processed in strip-pure pairs ((h%3) equal within each pair), ordered
# so v chunks (c0: heads 0-3, c1: 4-7, c2: 8-11) are needed progressively
HEAD_ORDER = [0, 3, 1, 4, 2, 5, 6, 9, 7, 10, 8, 11]
# qk chunk emission order: rounds 0-2 need q chunks {0,1}, k {4,5}
FC_ORDER = [0, 1, 4, 5, 2, 3, 6, 7]


def _build(bpc, use_bqk, use_bp):
    from contextlib import ExitStack

    import concourse.bacc as bacc
    import concourse.tile as tile
    from concourse import mybir

    f32 = mybir.dt.float32
    f32r = mybir.dt.float32r
    fp16 = mybir.dt.float16
    Alu = mybir.AluOpType
    Act = mybir.ActivationFunctionType

    nc = bacc.Bacc("TRN2", target_bir_lowering=False, debug=False,
                   num_devices=NCORES)

    x_d = nc.dram_tensor("x", [bpc, N, DIM], f32r, kind="ExternalInput").ap()
    wqk_d = nc.dram_tensor("wqk", [DIM, 8 * 128], f32r, kind="ExternalInput").ap()
    wv_d = nc.dram_tensor("wv", [DIM, DH], f32r, kind="ExternalInput").ap()
    wp_d = nc.dram_tensor("wp", [DH, DIM], f32r, kind="ExternalInput").ap()
    expb_d = nc.dram_tensor("expb", [128, 2, H, N], fp16, kind="ExternalInput").ap()
    ones_d = nc.dram_tensor("ones", [128, 128], f32r, kind="ExternalInput").ap()
    ident_d = nc.dram_tensor("ident", [128, 128], f32r, kind="ExternalInput").ap()
    if use_bqk:
        bqk_d = nc.dram_tensor("bqk", [128, 8], f32, kind="ExternalInput").ap()
    if use_bp:
        bp_d = nc.dram_tensor("bp", [DIM], f32, kind="ExternalInput").ap()
    y_d = nc.dram_tensor("y", [bpc, N, DIM], f32, kind="ExternalOutput").ap()

    NP = bpc // 2

    with tile.TileContext(nc) as tc, ExitStack() as ctx:
        consts = ctx.enter_context(tc.tile_pool(name="consts", bufs=1))
        sb_x = ctx.enter_context(tc.tile_pool(name="sb_x", bufs=2))
        sb_zT = ctx.enter_context(tc.tile_pool(name="sb_zT", bufs=2))
        sb_qkT = ctx.enter_context(tc.tile_pool(name="sb_qkT", bufs=2))
        sb_v = ctx.enter_context(tc.tile_pool(name="sb_v", bufs=2))
        sb_pt = ctx.enter_context(tc.tile_pool(name="sb_pt", bufs=3))
        sb_zb = ctx.enter_context(tc.tile_pool(name="sb_zb", bufs=2))
        sb_ot = ctx.enter_context(tc.tile_pool(name="sb_ot", bufs=2))
        sb_yb = ctx.enter_context(tc.tile_pool(name="sb_yb", bufs=2))
        sb_small = ctx.enter_context(tc.tile_pool(name="sb_small", bufs=3))
        ps_work = ctx.enter_context(tc.tile_pool(name="ps_work", bufs=2, space="PSUM"))
        ps_s = ctx.enter_context(tc.tile_pool(name="ps_s", bufs=2, space="PSUM"))
        ps_ot = ctx.enter_context(tc.tile_pool(name="ps_ot", bufs=2, space="PSUM"))
        ps_z = ctx.enter_context(tc.tile_pool(name="ps_z", bufs=2, space="PSUM"))

        x_tiles = {}

        def issue_x(p):
            t = sb_x.tile([128, 4, DIM], f32r, tag="x", bufs=2)
            nc.sync.dma_start(
                out=t.rearrange("q (e two) d -> q e two d", e=2),
                in_=x_d[2 * p:2 * p + 2].rearrange("e (q two) d -> q e two d",
                                                   q=128))
            x_tiles[p] = t

        # first pair's x before the big consts
        issue_x(0)

        # ---- constants (Activation DMA queue, first-use order) ----
        ident = consts.tile([128, 128], f32r)
        nc.scalar.dma_start(out=ident, in_=ident_d)
        wqk_sb = consts.tile([128, 4, 8 * 128], f32r)
        nc.scalar.dma_start(out=wqk_sb, in_=wqk_d.rearrange("(kc p) f -> p kc f", p=128))
        wv_sb = consts.tile([128, 4, DH], f32r)
        nc.gpsimd.dma_start(out=wv_sb, in_=wv_d.rearrange("(kc p) f -> p kc f", p=128))
        expb_sb = consts.tile([128, 2, H, N], fp16)
        nc.gpsimd.dma_start(out=expb_sb, in_=expb_d)
        ones_sb = consts.tile([128, 128], f32r)
        nc.gpsimd.dma_start(out=ones_sb, in_=ones_d)
        wp_sb = consts.tile([128, H, DIM], f32r)
        nc.gpsimd.dma_start(out=wp_sb, in_=wp_d.rearrange("(h p) f -> p h f", p=128))
        eps_t = consts.tile([128, 1], f32)
        nc.vector.memset(eps_t, EPS)
        if use_bqk:
            bqk_sb = consts.tile([128, 8], f32)
            nc.gpsimd.dma_start(out=bqk_sb, in_=bqk_d)
        if use_bp:
            bp_sb = consts.tile([128, 1, DIM], f32)
            nc.gpsimd.dma_start(out=bp_sb, in_=bp_d.partition_broadcast(128))

        # ---- LayerNorm pieces (emitted spread through the previous pair) ----
        def ln_stats(p, st, mvt):
            stats = sb_small.tile([128, 6], f32, tag="stats", bufs=5)
            nc.vector.bn_stats(stats, x_tiles[p][:, st, :])
            nc.vector.bn_aggr(mvt[:, st, :], stats)

        def ln_finish(mvt):
            sig = sb_small.tile([128, 4], f32, tag="sig")
            nc.scalar.activation(sig, mvt[:, :, 1], Act.Sqrt, bias=eps_t,
                                 scale=1.0)
            rsig = sb_small.tile([128, 4], f32, tag="rsig")
            nc.vector.reciprocal(rsig, sig)
            return rsig

        def ln_norm(p, st, mvt, rsig):
            x_t = x_tiles[p]
            nc.vector.tensor_scalar(out=x_t[:, st, :], in0=x_t[:, st, :],
                                    scalar1=mvt[:, st, 0:1],
                                    scalar2=rsig[:, st:st + 1],
                                    op0=Alu.subtract, op1=Alu.mult)

        def transposes(p):
            zT = sb_zT.tile([128, 4, 2 * N], f32r, tag="zT")
            x_t = x_tiles[p]
            for st in range(4):
                el, sl = st // 2, st % 2
                zT_ps = ps_work.tile([128, 512], f32r, tag="work")
                for kc in range(4):
                    nc.tensor.transpose(zT_ps[:, kc * 128:(kc + 1) * 128],
                                        x_t[:, st, kc * 128:(kc + 1) * 128],
                                        ident)
                off = el * N + sl * 128
                nc.scalar.activation(zT[:, :, off:off + 128],
                                     zT_ps.rearrange("p (kc t) -> p kc t", kc=4),
                                     Act.Copy)
            return zT

        def qk_prod(zT):
            qkT = sb_qkT.tile([128, 8, 2 * N], f32r, tag="qkT", bufs=1)
            for i, fc in enumerate(FC_ORDER):
                qk_ps = ps_work.tile([128, 512], f32, tag="work")
                for kc in range(4):
                    nc.tensor.matmul(qk_ps,
                                     lhsT=wqk_sb[:, kc, fc * 128:(fc + 1) * 128],
                                     rhs=zT[:, kc, :],
                                     start=(kc == 0), stop=(kc == 3))
                if i % 2 == 0:
                    nc.scalar.activation(qkT[:, fc, :], qk_ps, Act.Copy)
                else:
                    nc.vector.tensor_copy(out=qkT[:, fc, :], in_=qk_ps)
                if use_bqk:
                    nc.vector.tensor_scalar_add(
                        out=qkT[:, fc, :], in0=qkT[:, fc, :],
                        scalar1=bqk_sb[:, fc:fc + 1])
            return qkT

        def v_mm(zT, v_sb, etok, mc, c):
            v_ps = ps_work.tile([128, 512], f32, tag="work")
            for kc in range(4):
                nc.tensor.matmul(
                    v_ps,
                    lhsT=zT[:, kc, etok + mc * 128:etok + (mc + 1) * 128],
                    rhs=wv_sb[:, kc, c * 512:(c + 1) * 512],
                    start=(kc == 0), stop=(kc == 3))
            if (mc + c) % 2 == 0:
                nc.scalar.activation(v_sb[:, mc, c * 512:(c + 1) * 512], v_ps,
                                     Act.Copy)
            else:
                nc.vector.tensor_copy(out=v_sb[:, mc, c * 512:(c + 1) * 512],
                                      in_=v_ps)

        def s_round(qkT, etok, r):
            pt = sb_pt.tile([128, 2, 2, N], f32r, tag="pt")
            for mc in range(2):
                s_ps = ps_s.tile([128, 512], f32, tag="s")
                for hi in range(2):
                    h = HEAD_ORDER[2 * r + hi]
                    qc = h // 3
                    base = (h % 3) * KD
                    nc.tensor.matmul(
                        s_ps[:, hi * N:(hi + 1) * N],
                        lhsT=qkT[base:base + KD, 4 + qc,
                                 etok + mc * 128:etok + (mc + 1) * 128],
                        rhs=qkT[base:base + KD, qc, etok:etok + N],
                        start=True, stop=True)
                nc.scalar.activation(pt[:, mc],
                                     s_ps.rearrange("p (a n) -> p a n", a=2),
                                     Act.Exp)
                eng = nc.gpsimd if (r + mc) % 2 == 0 else nc.vector
                eng.tensor_tensor(out=pt[:, mc], in0=pt[:, mc],
                                  in1=expb_sb[:, mc, 2 * r:2 * r + 2, :],
                                  op=Alu.mult)
            return pt

        def zav_round(pt, v_sb, ot_sb, r):
            zb_ps = ps_z.tile([128, 512], f32, tag="zb")
            for mc in range(2):
                nc.tensor.matmul(zb_ps,
                                 lhsT=ones_sb,
                                 rhs=pt[:, mc, :, :].rearrange("p a n -> p (a n)"),
                                 start=(mc == 0), stop=(mc == 1))
            zr = sb_zb.tile([128, 2, N], f32, tag="zb")
            nc.vector.reciprocal_approx_fast(
                out=zr, in_=zb_ps.rearrange("p (a n) -> p a n", a=2))
            ot_ps = ps_ot.tile([128, 512], f32, tag="otp")
            for hi in range(2):
                h = HEAD_ORDER[2 * r + hi]
                for mc in range(2):
                    nc.tensor.matmul(
                        ot_ps[:, hi * N:(hi + 1) * N],
                        lhsT=v_sb[:, mc, h * 128:(h + 1) * 128],
                        rhs=pt[:, mc, hi, :],
                        start=(mc == 0), stop=(mc == 1))
            # GpSimd cannot read PSUM; normalize stays on DVE
            nc.vector.tensor_tensor(out=ot_sb[:, 2 * r:2 * r + 2, :],
                              in0=ot_ps.rearrange("p (a n) -> p a n", a=2),
                              in1=zr, op=Alu.mult)

        def proj(ot_sb, e):
            yb = sb_yb.tile([128, 2, DIM], f32, tag="yb")
            for nci in range(2):
                y_ps = ps_work.tile([128, 512], f32, tag="work")
                for slot in range(H):
                    nc.tensor.matmul(
                        y_ps,
                        lhsT=ot_sb[:, slot, nci * 128:(nci + 1) * 128],
                        rhs=wp_sb[:, HEAD_ORDER[slot], :],
                        start=(slot == 0), stop=(slot == H - 1))
                if use_bp:
                    nc.vector.tensor_tensor(out=yb[:, nci, :], in0=y_ps,
                                            in1=bp_sb[:, 0, :], op=Alu.add)
                else:
                    nc.scalar.activation(yb[:, nci, :], y_ps, Act.Copy)
            nc.sync.dma_start(
                out=y_d[e].rearrange("(q two) d -> q two d", q=128), in_=yb)

        # ---- prologue: pair 0's LN + transposes ----
        assert bpc % 2 == 0
        mv0 = sb_small.tile([128, 4, 2], f32, tag="mv")
        for st in range(4):
            ln_stats(0, st, mv0)
        rs0 = ln_finish(mv0)
        for st in range(4):
            ln_norm(0, st, mv0, rs0)
        zT = transposes(0)

        for p in range(NP):
            if p + 1 < NP:
                issue_x(p + 1)
                mv_n = sb_small.tile([128, 4, 2], f32, tag="mv")
                rs_holder = [None]
            qkT = qk_prod(zT)

            prev = None  # (ot_sb, e) of el0 awaiting proj
            for el in range(2):
                e = 2 * p + el
                etok = el * N
                v_sb = sb_v.tile([128, 2, DH], f32r, tag="v")
                ot_sb = sb_ot.tile([128, H, N], f32r, tag="ot")
                pts = {}

                def hook(i):
                    # spread next pair's LN through el0's rounds
                    if el != 0 or p + 1 >= NP:
                        return
                    if i < 4:
                        ln_stats(p + 1, i, mv_n)
                    elif i == 4:
                        rs_holder[0] = ln_finish(mv_n)
                        ln_norm(p + 1, 0, mv_n, rs_holder[0])
                        ln_norm(p + 1, 1, mv_n, rs_holder[0])
                    else:
                        ln_norm(p + 1, 2, mv_n, rs_holder[0])
                        ln_norm(p + 1, 3, mv_n, rs_holder[0])

                v_mm(zT, v_sb, etok, 0, 0)
                v_mm(zT, v_sb, etok, 1, 0)
                if prev is not None:
                    proj(*prev)
                    prev = None
                pts[0] = s_round(qkT, etok, 0)
                hook(0)
                v_mm(zT, v_sb, etok, 0, 1)
                pts[1] = s_round(qkT, etok, 1)
                hook(1)
                v_mm(zT, v_sb, etok, 1, 1)
                pts[2] = s_round(qkT, etok, 2)
                zav_round(pts.pop(0), v_sb, ot_sb, 0)
                hook(2)
                v_mm(zT, v_sb, etok, 0, 2)
                pts[3] = s_round(qkT, etok, 3)
                zav_round(pts.pop(1), v_sb, ot_sb, 1)
                hook(3)
                v_mm(zT, v_sb, etok, 1, 2)
                pts[4] = s_round(qkT, etok, 4)
                zav_round(pts.pop(2), v_sb, ot_sb, 2)
                hook(4)
                pts[5] = s_round(qkT, etok, 5)
                zav_round(pts.pop(3), v_sb, ot_sb, 3)
                hook(5)
                zav_round(pts.pop(4), v_sb, ot_sb, 4)
                zav_round(pts.pop(5), v_sb, ot_sb, 5)

                if el == 0:
                    prev = (ot_sb, e)
                else:
                    # next pair's transposes cover proj's wait for the last
                    # normalize
                    if p + 1 < NP:
                        zT = transposes(p + 1)
                    proj(ot_sb, e)

    nc.compile()
    return nc


def _prepare(x, gamma, beta, Wqkv, bqkv, Wproj, bproj, biases, bias_idxs):
    x = np.ascontiguousarray(np.asarray(x, dtype=np.float32))
    gamma = np.asarray(gamma, dtype=np.float32)
    beta = np.asarray(beta, dtype=np.float32)
    Wqkv = np.asarray(Wqkv, dtype=np.float32)
    bqkv = np.asarray(bqkv, dtype=np.float32)
    Wproj = np.asarray(Wproj, dtype=np.float32)
    bproj = np.asarray(bproj, dtype=np.float32)
    biases = np.asarray(biases, dtype=np.float32)
    bias_idxs = np.asarray(bias_idxs)

    s = np.float32(KD ** -0.5)
    Wg = Wqkv * gamma[:, None]
    bfull = beta @ Wqkv + bqkv
    Wr = Wg.reshape(DIM, H, 64 + D)
    br = bfull.reshape(H, 64 + D)
    # feature layout (see kernel comment): head h -> strip h%3; q in chunk
    # h//3, k in chunk 4 + h//3.
    wqk = np.zeros((DIM, 8, 128), dtype=np.float32)
    bqk = np.zeros((8, 128), dtype=np.float32)
    for h in range(H):
        qc, base = h // 3, (h % 3) * KD
        wqk[:, qc, base:base + KD] = Wr[:, h, 0:KD] * s
        wqk[:, 4 + qc, base:base + KD] = Wr[:, h, KD:2 * KD]
        bqk[qc, base:base + KD] = br[h, 0:KD] * s
        bqk[4 + qc, base:base + KD] = br[h, KD:2 * KD]
    wqk = np.ascontiguousarray(wqk.reshape(DIM, 8 * 128))
    wv = np.ascontiguousarray(Wr[:, :, 2 * KD:].reshape(DIM, DH))
    bv = br[:, 2 * KD:].reshape(DH)
    bp = bproj + bv @ Wproj
    expb = np.exp(biases[:, bias_idxs])  # [H, N, N]
    # token positions are interleaved 2-per-partition: pos (s*128+q) <-> token
    # 2q+s; permute both attention axes to match, then reorder heads to the
    # kernel's processing order
    perm = np.arange(N).reshape(128, 2).T.reshape(-1)  # pos -> token
    expb_p = expb[HEAD_ORDER][:, perm][:, :, perm]
    expb_t = np.ascontiguousarray(
        expb_p.reshape(H, 2, 128, N).transpose(2, 1, 0, 3)).astype(np.float16)

    use_bqk = bool(np.abs(bqk).max() > 0)
    use_bp = bool(np.abs(bp).max() > 0)
    bqk_t = np.ascontiguousarray(bqk.T)  # [128, 8]

    common = {"wqk": wqk, "wv": wv, "wp": np.ascontiguousarray(Wproj),
              "expb": expb_t, "ones": np.ones((128, 128), dtype=np.float32),
              "ident": np.eye(128, dtype=np.float32)}
    if use_bqk:
        common["bqk"] = bqk_t
    if use_bp:
        common["bp"] = np.ascontiguousarray(bp)
    in_maps = []
    for c in range(NCORES):
        m = dict(common)
        m["x"] = np.ascontiguousarray(x[c * BPC:(c + 1) * BPC])
        in_maps.append(m)
    return in_maps, use_bqk, use_bp


def run(inputs, trace=False, **run_kwargs):
    from concourse.bass_utils import run_bass_kernel_spmd

    in_maps, use_bqk, use_bp = _prepare(**inputs)
    key = (BPC, use_bqk, use_bp)
    if key not in _CACHE:
        _CACHE[key] = _build(*key)
    nc = _CACHE[key]
    res = run_bass_kernel_spmd(nc, in_maps, core_ids=list(range(NCORES)),
                               trace=trace, **run_kwargs)
    y = np.concatenate([res.results[c]["y"] for c in range(NCORES)], axis=0)
    return y, res


def kernel(**inputs):
    y, _ = run(inputs)
    return y
